# revision 18
# baseline (speedup 1.0000x reference)
"""Trainium2 Bass kernel for nn_CombinedLoss (surface loss + Tversky loss).

The reference computes a 4D (C,D,H,W) Euclidean distance transform of the
one-hot argmax mask per batch element, but because the EDT includes the
channel axis (C=3) the distance maps collapse analytically:

  * pos_d == 1 at every pos voxel (a zero channel-neighbor always exists at
    distance 1), so the (pos_d - 1) * pos term is identically zero.
  * neg_d at channel 1 (the only channel SurfaceLoss reads, idc=[1]) is
    sqrt(min(spatial_dist^2_to_cls1, 1)) == 1 at every voxel with cls != 1.

  => dist_maps[:, 1] == (argmax_c probs != 1), exactly (verified vs scipy EDT).

So the whole loss is elementwise work + global reductions:

  surface = mean(p1 * [argmax != 1])        over B*D*H*W voxels
  tversky = 1 - (tp + 1) / (0.5*(sum(p)+sum(t)) + 1),   tp = sum(p*t)

Inputs are shipped to the device as bf16. [argmax != 1] = 1[max(p0,p2) >= p1]
would pick up a one-sided bias from bf16 ties, so ties count 1/2 via
ind = 0.5 + 0.5*sign(max(p0,p2) - p1), giving ~3e-6 total relative error
(validated on the exact reference inputs on host):
  surface = (0.5*sum(p1) + 0.5*sum(p1*sign(m - p1))) / N_VOX

Work split per core (voxels are flattened and split evenly across 8 cores;
host does the final tiny reduction in f64):
  * DVE: m=max(p0,p2); d=m-p1; accumulate p1*s (s from ACT) via
    scalar_tensor_tensor(bypass,mult,accum_out).
  * ACT: s=sign(d); also issues the t-plane DMAs (both SP and ACT can
    trigger HWDGE, halving DMA trigger serialization).
  * PE:  tp via the diagonal trick (psa[128,129] += p_tile^T @ [t_tile|ones]
    over all channel/voxel tiles: diag = p*t partials, col 128 = sum(p));
    sum(t) (psb[1,387], baked ones columns subtracted on host) and sum(p1)
    (psc[1,512]) via ones-column stationary streams.
Raw Bass with standalone waits (this toolchain rejects instructions carrying
more than one attached sync-wait).
"""

import numpy as np
import ml_dtypes

import concourse.bass as bass
import concourse.mybir as mybir
from concourse.bass_utils import run_bass_kernel_spmd

N_CORES = 8
B, C, D, H, W = 2, 3, 64, 128, 128
N_VOX = B * D * H * W            # 2_097_152
VOX_PER_CORE = N_VOX // N_CORES  # 262_144
P = 128                          # partitions
NCH = 4                          # chunks per core
CW = VOX_PER_CORE // (P * NCH)   # 512 columns per chunk
TPC = CW // P                    # 4 PE tiles per chunk per channel
PW = C * CW                      # 1536 p-columns per chunk
TW = C * (CW + TPC)              # 1548 t-columns per chunk (ones baked in)
N_ONES = C * NCH * TPC * P       # total baked-ones contribution to psb: 6144

_CACHE = {}


def _build_module():
    from contextlib import ExitStack

    Alu = mybir.AluOpType
    Act = mybir.ActivationFunctionType
    f32 = mybir.dt.float32
    bf16 = mybir.dt.bfloat16

    nc = bass.Bass()
    p_in = nc.dram_tensor("p", [NCH, P, PW], bf16, kind="ExternalInput")
    t_in = nc.dram_tensor("t", [NCH, P, TW], bf16, kind="ExternalInput")
    s1_out = nc.dram_tensor("s1", [P, NCH], f32, kind="ExternalOutput")
    psa_out = nc.dram_tensor("psa", [P, P + 1], f32, kind="ExternalOutput")
    psbc_out = nc.dram_tensor("psbc", [1, 387 + CW], f32, kind="ExternalOutput")

    with (
        ExitStack() as ctx,
        nc.sbuf_tensor([P, NCH * PW], bf16) as p_sb,
        nc.sbuf_tensor([P, NCH * TW], bf16) as t_sb,
        nc.sbuf_tensor([P, CW], bf16) as m_sb,
        nc.sbuf_tensor([P, CW], bf16) as d_sb,
        nc.sbuf_tensor([P, CW], bf16) as s_sb,
        nc.sbuf_tensor([P, CW], bf16) as vj,
        nc.sbuf_tensor([P, NCH], f32) as s1_sb,
        nc.sbuf_tensor([P, P + 1], f32) as psa_sb,
        nc.sbuf_tensor([1, 387 + CW], f32) as psbc_sb,
        nc.psum_tensor([P, P + 1], f32) as psa,
        nc.psum_tensor([1, 387], f32) as psb,
        nc.psum_tensor([1, CW], f32) as psc,
        nc.semaphore() as v_sem,
        nc.semaphore() as a_sem,
        nc.semaphore() as pe_sem,
        nc.semaphore() as c_sem,
        nc.semaphore() as o1_sem,
        nc.semaphore() as o2_sem,
        nc.semaphore() as o3_sem,
        nc.Block() as block,
    ):
        p_sems = [ctx.enter_context(nc.semaphore(f"p_sem{i}")) for i in range(NCH)]
        t_sems = [ctx.enter_context(nc.semaphore(f"t_sem{i}")) for i in range(NCH)]

        def pp(ch, c):
            return p_sb[:, ch * PW + c * CW : ch * PW + (c + 1) * CW]

        def ptile(ch, c, i):
            off = ch * PW + c * CW + i * P
            return p_sb[:, off : off + P]

        def tblock(ch, c, i):
            off = ch * TW + c * (CW + TPC) + i * (P + 1)
            return t_sb[:, off : off + P + 1]

        @block.sync
        def _(sync):
            for ch in range(NCH):
                sync.dma_start(
                    p_sb[:, ch * PW : (ch + 1) * PW], p_in[ch]
                ).then_inc(p_sems[ch], 16)
            sync.wait_ge(v_sem, NCH * 3)
            sync.dma_start(s1_out[:], s1_sb[:]).then_inc(o1_sem, 16)
            sync.wait_ge(c_sem, 3)
            sync.dma_start(psa_out[:], psa_sb[:]).then_inc(o2_sem, 16)
            sync.dma_start(psbc_out[:], psbc_sb[:]).then_inc(o3_sem, 16)
            sync.wait_ge(o1_sem, 16)
            sync.wait_ge(o2_sem, 16)
            sync.wait_ge(o3_sem, 16)

        @block.scalar
        def _(scalar):
            for ch in range(NCH):
                scalar.dma_start(
                    t_sb[:, ch * TW : (ch + 1) * TW], t_in[ch]
                ).then_inc(t_sems[ch], 16)
            for ch in range(NCH):
                scalar.wait_ge(v_sem, 3 * ch + 2)
                if ch:
                    scalar.wait_ge(a_sem, ch)
                scalar.sign(s_sb[:], d_sb[:]).then_inc(a_sem, 1)

        @block.vector
        def _(vector):
            for ch in range(NCH):
                vector.wait_ge(p_sems[ch], 16)
                if ch:
                    vector.wait_ge(v_sem, 3 * ch)
                vector.tensor_tensor(
                    m_sb[:], pp(ch, 0), pp(ch, 2), Alu.max
                ).then_inc(v_sem, 1)
                vector.wait_ge(v_sem, 3 * ch + 1)
                vector.tensor_tensor(
                    d_sb[:], m_sb[:], pp(ch, 1), Alu.subtract
                ).then_inc(v_sem, 1)
                vector.wait_ge(a_sem, ch + 1)
                vector.scalar_tensor_tensor(
                    vj[:], pp(ch, 1), 0.0, s_sb[:], Alu.bypass, Alu.mult,
                    accum_out=s1_sb[:, ch : ch + 1],
                ).then_inc(v_sem, 1)
            # PSUM -> SBUF copies once PE is done
            vector.wait_ge(pe_sem, 3)
            vector.tensor_copy(psa_sb[:], psa[:]).then_inc(c_sem, 1)
            vector.tensor_copy(psbc_sb[:, :387], psb[:]).then_inc(c_sem, 1)
            vector.tensor_copy(psbc_sb[:, 387:], psc[:]).then_inc(c_sem, 1)

        @block.tensor
        def _(tensor):
            n_tp = NCH * C * TPC          # 48 tp matmuls
            n_st = NCH * 4                # 16 sum(t) matmuls (387 cols each)
            n_sp = NCH                    # 4 sum(p1) matmuls (512 cols each)
            k_tp = k_st = k_sp = 0
            ones_col = t_sb[:, P : P + 1]  # any baked ones column
            for ch in range(NCH):
                tensor.wait_ge(p_sems[ch], 16)
                tensor.wait_ge(t_sems[ch], 16)
                for c in range(C):
                    for i in range(TPC):
                        mm = nc.tensor.matmul(
                            psa[:],
                            ptile(ch, c, i),
                            tblock(ch, c, i),
                            start=(k_tp == 0),
                            stop=(k_tp == n_tp - 1),
                        )
                        if k_tp == n_tp - 1:
                            mm.then_inc(pe_sem, 1)
                        k_tp += 1
                for q in range(4):
                    off = ch * TW + q * 387
                    mm = nc.tensor.matmul(
                        psb[:],
                        ones_col,
                        t_sb[:, off : off + 387],
                        start=(k_st == 0),
                        stop=(k_st == n_st - 1),
                    )
                    if k_st == n_st - 1:
                        mm.then_inc(pe_sem, 1)
                    k_st += 1
                mm = nc.tensor.matmul(
                    psc[:],
                    ones_col,
                    pp(ch, 1),
                    start=(k_sp == 0),
                    stop=(k_sp == n_sp - 1),
                )
                if k_sp == n_sp - 1:
                    mm.then_inc(pe_sem, 1)
                k_sp += 1

    return nc


def _shard(probs, target):
    """f32 [B,C,D,H,W] x2 -> per-core bf16 arrays:
    p [NCH, P, C*CW] and t [NCH, P, C*(CW+TPC)] (ones columns baked in)."""
    pf = np.ascontiguousarray(probs.transpose(1, 0, 2, 3, 4)).reshape(C, N_VOX)
    tf = np.ascontiguousarray(target.transpose(1, 0, 2, 3, 4)).reshape(C, N_VOX)
    out = []
    for k in range(N_CORES):
        sl = slice(k * VOX_PER_CORE, (k + 1) * VOX_PER_CORE)
        pk = pf[:, sl].reshape(C, P, NCH, CW).transpose(2, 1, 0, 3)
        pk = np.ascontiguousarray(pk).astype(ml_dtypes.bfloat16)
        tk4 = tf[:, sl].reshape(C, P, NCH, TPC, P).transpose(2, 1, 0, 3, 4)
        tk = np.ones((NCH, P, C, TPC, P + 1), dtype=ml_dtypes.bfloat16)
        tk[..., :P] = tk4.astype(ml_dtypes.bfloat16)
        out.append(
            (
                pk.reshape(NCH, P, PW),
                np.ascontiguousarray(tk.reshape(NCH, P, TW)),
            )
        )
    return out


def _finalize(results):
    s1s = tp = sp = st = sp1 = 0.0
    for r in results:
        s1s += r["s1"].astype(np.float64).sum()
        psa = r["psa"].astype(np.float64)
        tp += np.diag(psa[:, :P]).sum()
        sp += psa[:, P].sum()
        psbc = r["psbc"].astype(np.float64)
        st += psbc[0, :387].sum() - N_ONES
        sp1 += psbc[0, 387:].sum()
    surface = 0.5 * (sp1 + s1s) / float(N_VOX)
    tversky = 1.0 - (tp + 1.0) / (0.5 * (sp + st) + 1.0)
    return np.float32(surface + tversky)


def kernel(probs: np.ndarray, target: np.ndarray) -> np.ndarray:
    probs = np.asarray(probs, dtype=np.float32)
    target = np.asarray(target, dtype=np.float32)

    if "nc" not in _CACHE:
        _CACHE["nc"] = _build_module()
    nc = _CACHE["nc"]

    shards = _shard(probs, target)
    in_maps = [{"p": p, "t": t} for p, t in shards]
    res = run_bass_kernel_spmd(nc, in_maps, core_ids=list(range(N_CORES)))
    return _finalize(res.results)
